# revision 28
# baseline (speedup 1.0000x reference)
"""Trainium2 Bass kernel for nn_Attention (B=4, N=2048, D=1024, H=16, Dh=64).

Distribution over 8 NeuronCores: 4-way data parallel on batch x 2-way tensor
parallel on heads (8 heads / 512 inner dims per core). Each core computes a
partial output projection; the host sums the two head-group partials per batch
and adds bo.

Per-core dataflow (v3 — chain-safe interleaved single pass):
  A) x-LN stats on DVE, rstd = exp(-0.5*ln(var+eps)) on ACT (keeps the whole
     kernel in the natural_log_exp table -> zero mid-kernel table reloads),
     LN apply as one DVE stt, X^T via DMA XBAR transpose (off the PE).
     Q/K/V token-major matmuls; qk-LN via Pool squares + DVE per-head
     reduces + one joint ACT ln/exp rstd per tile; Q/K jointly
     DMA-transposed into qkt.
  B) attention in S^T layout: S^T = K @ Q^T with per-chunk causal q-slicing,
     exp on ACT over kc-pair tiles, causal mask as one batched
     [128,2,128] tri-multiply per diagonal pair on DVE, P^T@V into [65,512]
     PSUM with fused ones-column denominator, PV software-pipelined one
     kc-pair behind S. Normalize: DVE reciprocal straight from PSUM ->
     f32 partition_broadcast (Pool) -> DVE stt.
  C) out-proj per 128-token tile, split into a matmul unit and a lagged
     finish unit (copies + DMA) so no engine queue ever holds a
     not-yet-ready instruction.
Units are emitted so that every instruction is ready (or nearly) when its
in-order engine queue reaches it: cross-engine chains are split into stages
lagged by >= 1 unit, and A(g+1)/C(qc-1) units interleave between B(qc)
units to keep the PE continuously fed (p-state stays at full clock).
"""

import os
import sys
import types

import numpy as np
import ml_dtypes

B, N, D = 4, 2048, 1024
H, Dh = 16, 64
HL, IL = 8, 512          # local heads / local inner per core
SCALE = 8.0 / Dh
EPS = 1e-5
N_CORES = 8
TC = N // 128            # 16 token chunks
TG = N // 512            # 4 token groups
QC = N // 512            # 4 query chunks


def _install_ntff_hook_shim():
    """The agent image's antenv lacks axon_hooks; recreate it so
    run_bass_kernel_spmd(trace=True) can profile via libaxon_pjrt."""
    try:
        if "antenv.axon_hooks" in sys.modules:
            return True
        import antenv

        mod = types.ModuleType("antenv.axon_hooks")
        _state = {"hook": None}
        mod.set_axon_ntff_profile_hook = lambda h: _state.__setitem__("hook", h)
        mod.get_axon_ntff_profile_hook = lambda: _state["hook"]
        sys.modules["antenv.axon_hooks"] = mod
        antenv.axon_hooks = mod

        from trn_agent_boot.trn_boot import _ntff_profile_via_ctypes

        so = "/opt/axon/libaxon_pjrt.so"
        if os.path.exists(so):
            mod.set_axon_ntff_profile_hook(_ntff_profile_via_ctypes(so))
        return True
    except Exception:
        return False


_BUILD_CACHE = {}


def _build_program(flags):
    """Build + compile the per-core Bass program. flags: (use_bias_q/k/v,
    use_qnw, use_qnb, use_knw, use_knb) booleans for the general paths."""
    if flags in _BUILD_CACHE:
        return _BUILD_CACHE[flags]

    import concourse.bass as bass
    import concourse.bacc as bacc
    import concourse.mybir as mybir
    import concourse.tile as tile

    use_bq, use_bk, use_bv, use_qnw, use_qnb, use_knw, use_knb = flags
    f32 = mybir.dt.float32
    bf16 = mybir.dt.bfloat16
    AF = mybir.ActivationFunctionType
    MUL = mybir.AluOpType.mult
    SUB = mybir.AluOpType.subtract

    nc = bacc.Bacc("TRN2", target_bir_lowering=False, debug=False,
                   num_devices=N_CORES)

    x_d = nc.dram_tensor("x", [N, D], f32, kind="ExternalInput")
    # weights pre-rearranged on host to [128, ...] so device DMA loads are
    # fully contiguous (128 descriptors instead of 1024)
    wq_d = nc.dram_tensor("wq", [128, 8 * IL], bf16, kind="ExternalInput")
    wk_d = nc.dram_tensor("wk", [128, 8 * IL], bf16, kind="ExternalInput")
    wv_d = nc.dram_tensor("wv", [128, 8 * IL], bf16, kind="ExternalInput")
    wo_d = nc.dram_tensor("wo", [128, 4 * D], bf16, kind="ExternalInput")
    out_d = nc.dram_tensor("out", [N, D], bf16, kind="ExternalOutput")
    extra_d = {}
    for name, used in (("bq", use_bq), ("bk", use_bk), ("bv", use_bv),
                       ("qnw", use_qnw), ("qnb", use_qnb),
                       ("knw", use_knw), ("knb", use_knb)):
        if used:
            extra_d[name] = nc.dram_tensor(name, [1, IL], f32,
                                           kind="ExternalInput")

    with tile.TileContext(nc) as tc:
        with tc.tile_pool(name="const", bufs=1) as constp, \
             tc.tile_pool(name="w", bufs=1) as wpool, \
             tc.tile_pool(name="x", bufs=8) as xpool, \
             tc.tile_pool(name="xr", bufs=4) as xrpool, \
             tc.tile_pool(name="xt", bufs=2) as xtpool, \
             tc.tile_pool(name="qkln", bufs=4) as qklnp, \
             tc.tile_pool(name="qkt", bufs=1) as qktp, \
             tc.tile_pool(name="v", bufs=1) as vpool, \
             tc.tile_pool(name="ot", bufs=1) as otpool, \
             tc.tile_pool(name="pt", bufs=8) as ptpool, \
             tc.tile_pool(name="osb", bufs=2) as osbp, \
             tc.tile_pool(name="small", bufs=4) as small, \
             tc.tile_pool(name="ps", bufs=1, space="PSUM") as psp:

            # Preload the one ACT table that covers every function we use
            # (exp, ln, identity, copy) so the compiler's per-activation
            # greedy table choice never reloads mid-kernel.
            from concourse.hw_specs import get_activation_tables
            _tabs = list(get_activation_tables(nc.m.arch))
            _tab_id = _tabs.index("natural_log_exp_and_others")
            nc.scalar.add_instruction(mybir.InstLoadActFuncSet(
                name=nc.get_next_instruction_name(),
                act_func_set_id=_tab_id, ins=[], outs=[]))

            # ---- constants ----
            eps_t = constp.tile([128, 1], f32, tag="eps")
            nc.vector.memset(eps_t, EPS)
            # causal triangle: keep where q-col >= key-row
            tri = constp.tile([128, 128], bf16, tag="tri")
            nc.gpsimd.memset(tri, 1.0)
            nc.gpsimd.affine_select(
                out=tri, in_=tri, compare_op=mybir.AluOpType.is_ge,
                fill=0.0, base=0, channel_multiplier=-1, pattern=[[1, 128]])
            onesc = None
            if use_bq or use_bk or use_bv:
                onesc_f = small.tile([1, 128], f32, tag="onescf", bufs=1)
                nc.vector.memset(onesc_f, 1.0)
                onesc = constp.tile([1, 128], bf16, tag="onesc")
                nc.vector.tensor_copy(onesc, onesc_f)

            extra_sb = {}
            for name in ("bq", "bk", "bv"):
                if name in extra_d:
                    t = constp.tile([1, IL], bf16, tag=name)
                    tf = constp.tile([1, IL], f32, tag=name + "f")
                    nc.sync.dma_start(out=tf, in_=extra_d[name].ap())
                    nc.vector.tensor_copy(t, tf)
                    extra_sb[name] = t
            for name in ("qnw", "qnb", "knw", "knb"):
                if name in extra_d:
                    row = constp.tile([1, IL], f32, tag=name + "r")
                    nc.sync.dma_start(out=row, in_=extra_d[name].ap())
                    t = constp.tile([128, IL], f32, tag=name)
                    nc.gpsimd.partition_broadcast(t, row)
                    extra_sb[name] = t

            # ---- persistent tiles ----
            # qkt_g: chunks 0-3 = Q^T (dim1=hp, partition=sub*64+dh),
            #        chunks 4-7 = K^T
            qkt_g = [qktp.tile([128, 8, 512], bf16, tag=f"qkt{g}",
                               name=f"qkt_g{g}") for g in range(TG)]
            v_g = [vpool.tile([128, 4, HL, 65], bf16, tag=f"v{g}",
                              name=f"v_g{g}") for g in range(TG)]
            for g in range(TG):
                nc.gpsimd.memset(v_g[g][:, :, :, 64:65], 1.0)
            ot_g = [otpool.tile([128, 4, 512], bf16, tag=f"ot{g}",
                                name=f"ot_g{g}") for g in range(QC)]

            wq_sb = wpool.tile([128, 8, IL], bf16, tag="wq")
            wk_sb = wpool.tile([128, 8, IL], bf16, tag="wk")
            wv_sb = wpool.tile([128, 8, IL], bf16, tag="wv")
            wo_sb = wpool.tile([128, 4, D], bf16, tag="wo")
            w_of = {"q": wq_sb, "k": wk_sb, "v": wv_sb}

            state = {}

            # =================== A units ===================
            def x_load(g, t):
                x_t = xpool.tile([128, D], f32, tag="x", bufs=8,
                                 name=f"x_{g}_{t}")
                tci = g * 4 + t
                nc.sync.dma_start(
                    out=x_t, in_=x_d.ap()[tci * 128:(tci + 1) * 128, :])
                state[("x", g, t)] = x_t

            def a_ln_stats(g, t):
                if g + 2 < TG:
                    x_load(g + 2, t)
                x_t = state[("x", g, t)]
                bn = small.tile([128, 2, 6], f32, tag="bn", bufs=2,
                                name=f"bn_{g}_{t}")
                nc.vector.bn_stats(bn[:, 0, :], x_t[:, 0:512])
                nc.vector.bn_stats(bn[:, 1, :], x_t[:, 512:1024])
                mv = small.tile([128, 2], f32, tag="mv", bufs=3,
                                name=f"mv_{g}_{t}")
                nc.vector.bn_aggr(mv, bn)
                state[("mv", g, t)] = mv

            def a_ln_apply(g, t):
                x_t = state.pop(("x", g, t))
                mv = state.pop(("mv", g, t))
                # rstd = exp(-0.5 * ln(var + eps))
                nc.scalar.activation(out=mv[:, 1:2], in_=mv[:, 1:2],
                                     func=AF.Ln, bias=eps_t, scale=1.0)
                nc.scalar.activation(out=mv[:, 1:2], in_=mv[:, 1:2],
                                     func=AF.Exp, scale=-0.5)
                x_r = xrpool.tile([128, D], bf16, tag="xr", bufs=4,
                                  name=f"xr_{g}_{t}")
                # x_r = (x - mean) * rstd in one DVE op
                nc.vector.scalar_tensor_tensor(
                    out=x_r, in0=x_t, scalar=mv[:, 0:1], op0=SUB,
                    in1=mv[:, 1:2].broadcast_to([128, D]), op1=MUL)
                if t == 0:
                    state[("xt", g)] = xtpool.tile(
                        [128, 8, 512], bf16, tag="xt", bufs=2,
                        name=f"xt_{g}")
                nc.sync.dma_start(
                    out=state[("xt", g)][:, :, t * 128:(t + 1) * 128],
                    in_=x_r, transpose=True)

            def a_mm(g, t, proj):
                xt = state[("xt", g)]
                ps = psp.tile([128, 512], f32, tag="acc", bufs=2,
                              name=f"ps_{proj}_{g}_{t}")
                bias_sb = extra_sb.get("b" + proj)
                for dc in range(8):
                    nc.tensor.matmul(
                        ps, lhsT=xt[:, dc, t * 128:(t + 1) * 128],
                        rhs=w_of[proj][:, dc, :],
                        start=(dc == 0),
                        stop=(dc == 7 and bias_sb is None))
                if bias_sb is not None:
                    nc.tensor.matmul(ps, lhsT=onesc, rhs=bias_sb,
                                     start=False, stop=True)
                if proj == "v":
                    nc.vector.tensor_copy(
                        v_g[g][:, t, :, 0:64],
                        ps.rearrange("p (h d) -> p h d", h=HL))
                    return
                # free the PSUM slot in the same unit
                qraw = small.tile([128, 512], bf16, tag="qraw", bufs=4,
                                  name=f"qraw_{proj}_{g}_{t}")
                nc.vector.tensor_copy(qraw, ps)
                state[("qraw", proj, g, t)] = qraw

            def a_chain1(g, t):
                ss = small.tile([128, 2, HL], f32, tag="ss", bufs=2,
                                name=f"ss_{g}_{t}")
                for half, proj in enumerate(("q", "k")):
                    qraw = state[("qraw", proj, g, t)]
                    sq = small.tile([128, 512], bf16, tag="sq", bufs=2,
                                    name=f"sq_{proj}_{g}_{t}")
                    nc.gpsimd.tensor_mul(sq, qraw, qraw)
                    nc.vector.reduce_sum(
                        ss[:, half, :],
                        sq.rearrange("p (h d) -> p h d", h=HL),
                        axis=mybir.AxisListType.X)
                state[("ss", g, t)] = ss

            def a_chain2(g, t):
                ss = state.pop(("ss", g, t))
                # joint rstd for q and k: exp(-0.5*ln(ss/Dh + eps))
                nc.scalar.activation(out=ss, in_=ss, func=AF.Ln,
                                     bias=eps_t, scale=1.0 / Dh)
                nc.scalar.activation(out=ss, in_=ss, func=AF.Exp,
                                     scale=-0.5)
                qkln = qklnp.tile([128, 2, 512], bf16, tag="qkln", bufs=4,
                                  name=f"qkln_{g}_{t}")
                for half, proj in enumerate(("q", "k")):
                    qraw = state.pop(("qraw", proj, g, t))
                    nc.vector.scalar_tensor_tensor(
                        out=qkln[:, half, :].rearrange(
                            "p (h d) -> p h d", h=HL),
                        in0=qraw.rearrange("p (h d) -> p h d", h=HL),
                        scalar=1.0,
                        in1=ss[:, half, :].broadcast_to([128, HL, Dh]),
                        op0=MUL, op1=MUL)
                    if extra_sb.get(proj + "nw") is not None:
                        nc.vector.tensor_mul(
                            qkln[:, half, :], qkln[:, half, :],
                            extra_sb[proj + "nw"])
                    if extra_sb.get(proj + "nb") is not None:
                        nc.vector.tensor_add(
                            qkln[:, half, :], qkln[:, half, :],
                            extra_sb[proj + "nb"])
                nc.sync.dma_start(
                    out=qkt_g[g][:, :, t * 128:(t + 1) * 128],
                    in_=qkln.rearrange("p a b -> p (a b)"),
                    transpose=True)

            def units_a_all():
                # one globally software-pipelined A stream over all 16
                # tiles: stats/apply run ~1-2 tiles ahead of the matmuls,
                # chain2 one tile behind, so every instruction is ready
                # when its in-order engine queue reaches it and there are
                # no group-boundary stalls.
                seq = [(g, t) for g in range(TG) for t in range(4)]
                n = len(seq)
                S = lambda i: (lambda: a_ln_stats(*seq[i]))
                A_ = lambda i: (lambda: a_ln_apply(*seq[i]))
                Q = lambda i: (lambda: a_mm(*seq[i], "q"))
                K = lambda i: (lambda: a_mm(*seq[i], "k"))
                V = lambda i: (lambda: a_mm(*seq[i], "v"))
                C1 = lambda i: (lambda: a_chain1(*seq[i]))
                C2 = lambda i: (lambda: a_chain2(*seq[i]))
                us = [S(0), S(1), A_(0)]
                for i in range(n):
                    if i + 2 < n:
                        us.append(S(i + 2))
                    if i + 1 < n:
                        us.append(A_(i + 1))
                    us.extend([Q(i), K(i), V(i), C1(i)])
                    if i >= 1:
                        us.append(C2(i - 1))
                us.append(C2(n - 1))
                return us

            # =================== B units ===================
            def emit_pv(qc, hp, hold):
                pts, kc0 = hold["prev"]
                nkc = 4 * (qc + 1)
                for sub in range(2):
                    h = 2 * hp + sub
                    for i in range(2):
                        kc = kc0 + i
                        d = kc * 128 - qc * 512
                        q0c = d if d > 0 else 0
                        nc.tensor.matmul(
                            hold["ps_o"][sub][:, q0c:512],
                            lhsT=v_g[kc // 4][:, kc % 4, h, :],
                            rhs=pts[sub][:, i, q0c:512],
                            start=(kc == 0), stop=(kc == nkc - 1))

            def b_unit(qc, hp, kcg, hold):
                if kcg == 0:
                    hold["ps_o"] = [
                        psp.tile([65, 512], f32, tag="o", bufs=2,
                                 name=f"o_{qc}_{hp}_{s}") for s in range(2)]
                    hold["prev"] = None
                kc0 = 2 * kcg
                d0 = kc0 * 128 - qc * 512     # >= 0 iff diagonal pair
                q0p = 256 if d0 >= 256 else 0
                pts = []
                for sub in range(2):
                    r0 = 64 * sub
                    ps_s = psp.tile([128, 2, 512], f32, tag="s", bufs=2,
                                    name=f"s_{qc}_{hp}_{kcg}_{sub}")
                    for i in range(2):
                        kc = kc0 + i
                        d = kc * 128 - qc * 512
                        q0c = d if d > 0 else 0
                        nc.tensor.matmul(
                            ps_s[:, i, q0c:512],
                            lhsT=qkt_g[kc // 4][
                                r0:r0 + 64, 4 + hp,
                                (kc % 4) * 128:(kc % 4 + 1) * 128],
                            rhs=qkt_g[qc][r0:r0 + 64, hp, q0c:512],
                            start=True, stop=True)
                    pt = ptpool.tile([128, 2, 512], bf16, tag="pt", bufs=8,
                                     name=f"pt_{qc}_{hp}_{kcg}_{sub}")
                    nc.scalar.activation(
                        out=pt[:, :, q0p:512], in_=ps_s[:, :, q0p:512],
                        func=AF.Exp, scale=SCALE)
                    if d0 >= 0:
                        # batched tri-mask over both diag chunks of the pair
                        pt_ap = bass.AP(
                            tensor=pt.tensor, offset=pt.offset + d0,
                            ap=[pt.ap[0], [512 + 128, 2], [1, 128]])
                        tri_b = bass.AP(
                            tensor=tri.tensor, offset=tri.offset,
                            ap=[tri.ap[0], [0, 2], [1, 128]])
                        nc.vector.tensor_mul(pt_ap, pt_ap, tri_b)
                    pts.append(pt)
                if hold["prev"] is not None:
                    emit_pv(qc, hp, hold)
                hold["prev"] = (pts, kc0)

            def b_tail_a(qc, hp, hold):
                # last PV + copy out of PSUM (releases the ps_o ring fast)
                emit_pv(qc, hp, hold)
                for sub in range(2):
                    ps_o = hold["ps_o"][sub]
                    rf = small.tile([1, 512], f32, tag="rf", bufs=4,
                                    name=f"rf_{qc}_{hp}_{sub}")
                    nc.vector.tensor_copy(rf, ps_o[64:65, :])
                    ob = small.tile([64, 512], bf16, tag="ob", bufs=6,
                                    name=f"ob_{qc}_{hp}_{sub}")
                    nc.vector.tensor_copy(ob, ps_o[0:64, :])
                    state[("tail", qc, hp, sub)] = (rf, ob)

            def b_tail_b(qc, hp):
                # lagged normalize: everything reads SBUF, nothing blocks
                for sub in range(2):
                    r0 = 64 * sub
                    rf, ob = state.pop(("tail", qc, hp, sub))
                    nc.vector.reciprocal_approx_fast(rf, rf)
                    rb = small.tile([128, 512], f32, tag="rb", bufs=2,
                                    name=f"rb_{qc}_{hp}_{sub}")
                    nc.gpsimd.partition_broadcast(rb, rf)
                    nc.vector.scalar_tensor_tensor(
                        out=ot_g[qc][r0:r0 + 64, hp, :],
                        in0=ob, scalar=1.0, in1=rb[0:64, :],
                        op0=MUL, op1=MUL)

            def units_b(qc):
                us = []
                pend = []
                for hp in range(4):
                    hold = {}
                    for kcg in range(2 * (qc + 1)):
                        us.append(
                            lambda qc=qc, hp=hp, kcg=kcg, hold=hold:
                            b_unit(qc, hp, kcg, hold))
                        if pend and kcg == 1:
                            us.append(pend.pop(0))
                    us.append(lambda qc=qc, hp=hp, hold=hold:
                              b_tail_a(qc, hp, hold))
                    pend.append(lambda qc=qc, hp=hp: b_tail_b(qc, hp))
                us.extend(pend)
                return us

            # =================== C units ===================
            def c_mm(qc, t):
                tci = qc * 4 + t
                osb = osbp.tile([128, 2, 512], bf16, tag="osb", bufs=2,
                                name=f"osb_{tci}")
                pss = []
                for dch in range(2):
                    ps = psp.tile([128, 512], f32, tag="acc", bufs=2,
                                  name=f"ps_c_{tci}_{dch}")
                    for m in range(4):
                        nc.tensor.matmul(
                            ps, lhsT=ot_g[qc][:, m, t * 128:(t + 1) * 128],
                            rhs=wo_sb[:, m, dch * 512:(dch + 1) * 512],
                            start=(m == 0), stop=(m == 3))
                    pss.append(ps)
                state[("c", tci)] = (osb, pss)

            def c_fin(qc, t):
                tci = qc * 4 + t
                osb, pss = state.pop(("c", tci))
                nc.vector.tensor_copy(osb[:, 0, :], pss[0])
                nc.vector.tensor_copy(osb[:, 1, :], pss[1])
                nc.sync.dma_start(
                    out=out_d.ap()[tci * 128:(tci + 1) * 128, :],
                    in_=osb.rearrange("p a b -> p (a b)"))

            def units_c(qc):
                us = []
                for t in range(4):
                    us.append(lambda qc=qc, t=t: c_mm(qc, t))
                    us.append(lambda qc=qc, t=t: c_fin(qc, t))
                return us

            # =================== schedule ===================
            x_t00 = xpool.tile([128, D], f32, tag="x", bufs=8,
                               name="x_0_0")
            nc.sync.dma_start(out=x_t00[:, 0:512],
                              in_=x_d.ap()[0:128, 0:512])
            nc.sync.dma_start(out=x_t00[:, 512:1024],
                              in_=x_d.ap()[0:128, 512:1024])
            state[("x", 0, 0)] = x_t00
            x_load(0, 1)
            nc.sync.dma_start(out=wq_sb, in_=wq_d.ap())
            x_load(0, 2)
            x_load(0, 3)
            nc.sync.dma_start(out=wk_sb, in_=wk_d.ap())
            x_load(1, 0)
            x_load(1, 1)
            nc.sync.dma_start(out=wv_sb, in_=wv_d.ap())
            x_load(1, 2)
            x_load(1, 3)
            nc.sync.dma_start(out=wo_sb, in_=wo_d.ap())

            for u in units_a_all():
                u()

            def interleave(primary, fillers, front=0.85):
                """Emit primary units with fillers spread between them,
                front-biased so fillers finish by `front` fraction of the
                primary stream."""
                np_, nf = len(primary), len(fillers)
                fi = 0
                for j, u in enumerate(primary):
                    u()
                    want = min(nf, int(nf * (j + 1) / max(1, front * np_)))
                    while fi < want:
                        fillers[fi]()
                        fi += 1
                while fi < nf:
                    fillers[fi]()
                    fi += 1

            interleave(units_b(0), [])
            interleave(units_b(1), units_c(0))
            interleave(units_b(2), units_c(1))
            interleave(units_b(3), units_c(2), front=1.0)
            for u in units_c(3):
                u()

    nc.compile()
    _BUILD_CACHE[flags] = nc
    return nc


def kernel(**inputs):
    x = np.ascontiguousarray(np.asarray(inputs["x"], np.float32))
    ln_w = np.asarray(inputs["ln_w"], np.float32)
    ln_b = np.asarray(inputs["ln_b"], np.float32)
    Wq = np.asarray(inputs["Wq"], np.float32)
    Wk = np.asarray(inputs["Wk"], np.float32)
    Wv = np.asarray(inputs["Wv"], np.float32)
    qn_w = np.asarray(inputs["qn_w"], np.float32)
    qn_b = np.asarray(inputs["qn_b"], np.float32)
    kn_w = np.asarray(inputs["kn_w"], np.float32)
    kn_b = np.asarray(inputs["kn_b"], np.float32)
    Wo = np.asarray(inputs["Wo"], np.float32)
    bo = np.asarray(inputs["bo"], np.float32)

    # ---- host-side weight folding ----
    def fold(W):
        return ln_w[:, None] * W, ln_b @ W

    W1q, bq = fold(Wq)
    W1k, bk = fold(Wk)
    W1v, bv = fold(Wv)

    def center(W, b):
        W3 = W.reshape(D, H, Dh)
        W3 = W3 - W3.mean(-1, keepdims=True)
        b3 = b.reshape(H, Dh)
        b3 = b3 - b3.mean(-1, keepdims=True)
        return np.ascontiguousarray(W3.reshape(D, H * Dh)), b3.reshape(H * Dh)

    W1q, bq = center(W1q, bq)
    W1k, bk = center(W1k, bk)

    flags = (
        bool(np.any(bq)), bool(np.any(bk)), bool(np.any(bv)),
        not np.all(qn_w == 1.0), bool(np.any(qn_b)),
        not np.all(kn_w == 1.0), bool(np.any(kn_b)),
    )
    nc = _build_program(flags)

    def prearrange_w(W, nchunk):
        # [nchunk*128, F] -> [128, nchunk*F] with w_sb[p, c, f] = W[c*128+p, f]
        Wb = W.astype(ml_dtypes.bfloat16)
        F = Wb.shape[1]
        return np.ascontiguousarray(
            Wb.reshape(nchunk, 128, F).transpose(1, 0, 2).reshape(
                128, nchunk * F))

    wo_bf = Wo.astype(ml_dtypes.bfloat16)
    in_maps = []
    for c in range(N_CORES):
        b, g = c // 2, c % 2
        sl = slice(IL * g, IL * (g + 1))
        m = {
            "x": x[b],
            "wq": prearrange_w(W1q[:, sl], 8),
            "wk": prearrange_w(W1k[:, sl], 8),
            "wv": prearrange_w(W1v[:, sl], 8),
            "wo": prearrange_w(Wo[sl, :], 4),
        }
        if flags[0]:
            m["bq"] = np.ascontiguousarray(bq[None, sl])
        if flags[1]:
            m["bk"] = np.ascontiguousarray(bk[None, sl])
        if flags[2]:
            m["bv"] = np.ascontiguousarray(bv[None, sl])
        if flags[3]:
            m["qnw"] = np.ascontiguousarray(np.tile(qn_w, HL)[None, :])
        if flags[4]:
            m["qnb"] = np.ascontiguousarray(np.tile(qn_b, HL)[None, :])
        if flags[5]:
            m["knw"] = np.ascontiguousarray(np.tile(kn_w, HL)[None, :])
        if flags[6]:
            m["knb"] = np.ascontiguousarray(np.tile(kn_b, HL)[None, :])
        in_maps.append(m)

    from concourse.bass_utils import run_bass_kernel_spmd

    trace = _install_ntff_hook_shim() and \
        os.environ.get("KERNEL_NO_TRACE", "0") != "1"
    try:
        res = run_bass_kernel_spmd(
            nc, in_maps, core_ids=list(range(N_CORES)), trace=trace)
    except Exception:
        if not trace:
            raise
        res = run_bass_kernel_spmd(
            nc, in_maps, core_ids=list(range(N_CORES)), trace=False)
    globals()["LAST_RESULT"] = res
    if res.exec_time_ns is not None:
        print(f"HW exec time: {res.exec_time_ns} ns")

    out = np.zeros((B, N, D), np.float32)
    for b in range(B):
        out[b] = (np.asarray(res.results[2 * b]["out"], np.float32) +
                  np.asarray(res.results[2 * b + 1]["out"], np.float32))
    out += bo
    return out


# revision 29
# speedup vs baseline: 1.0046x; 1.0046x over previous
"""Trainium2 Bass kernel for nn_Attention (B=4, N=2048, D=1024, H=16, Dh=64).

Distribution over 8 NeuronCores: 4-way data parallel on batch x 2-way tensor
parallel on heads (8 heads / 512 inner dims per core). Each core computes a
partial output projection; the host sums the two head-group partials per batch
and adds bo.

Per-core dataflow (v3 — chain-safe interleaved single pass):
  A) x-LN stats on DVE, rstd = exp(-0.5*ln(var+eps)) on ACT (keeps the whole
     kernel in the natural_log_exp table -> zero mid-kernel table reloads),
     LN apply as one DVE stt, X^T via DMA XBAR transpose (off the PE).
     Q/K/V token-major matmuls; qk-LN via Pool squares + DVE per-head
     reduces + one joint ACT ln/exp rstd per tile; Q/K jointly
     DMA-transposed into qkt.
  B) attention in S^T layout: S^T = K @ Q^T with per-chunk causal q-slicing,
     exp on ACT over kc-pair tiles, causal mask as one batched
     [128,2,128] tri-multiply per diagonal pair on DVE, P^T@V into [65,512]
     PSUM with fused ones-column denominator, PV software-pipelined one
     kc-pair behind S. Normalize: DVE reciprocal straight from PSUM ->
     f32 partition_broadcast (Pool) -> DVE stt.
  C) out-proj per 128-token tile, split into a matmul unit and a lagged
     finish unit (copies + DMA) so no engine queue ever holds a
     not-yet-ready instruction.
Units are emitted so that every instruction is ready (or nearly) when its
in-order engine queue reaches it: cross-engine chains are split into stages
lagged by >= 1 unit, and A(g+1)/C(qc-1) units interleave between B(qc)
units to keep the PE continuously fed (p-state stays at full clock).
"""

import os
import sys
import types

import numpy as np
import ml_dtypes

B, N, D = 4, 2048, 1024
H, Dh = 16, 64
HL, IL = 8, 512          # local heads / local inner per core
SCALE = 8.0 / Dh
EPS = 1e-5
N_CORES = 8
TC = N // 128            # 16 token chunks
TG = N // 512            # 4 token groups
QC = N // 512            # 4 query chunks


def _install_ntff_hook_shim():
    """The agent image's antenv lacks axon_hooks; recreate it so
    run_bass_kernel_spmd(trace=True) can profile via libaxon_pjrt."""
    try:
        if "antenv.axon_hooks" in sys.modules:
            return True
        import antenv

        mod = types.ModuleType("antenv.axon_hooks")
        _state = {"hook": None}
        mod.set_axon_ntff_profile_hook = lambda h: _state.__setitem__("hook", h)
        mod.get_axon_ntff_profile_hook = lambda: _state["hook"]
        sys.modules["antenv.axon_hooks"] = mod
        antenv.axon_hooks = mod

        from trn_agent_boot.trn_boot import _ntff_profile_via_ctypes

        so = "/opt/axon/libaxon_pjrt.so"
        if os.path.exists(so):
            mod.set_axon_ntff_profile_hook(_ntff_profile_via_ctypes(so))
        return True
    except Exception:
        return False


_BUILD_CACHE = {}


def _build_program(flags):
    """Build + compile the per-core Bass program. flags: (use_bias_q/k/v,
    use_qnw, use_qnb, use_knw, use_knb) booleans for the general paths."""
    if flags in _BUILD_CACHE:
        return _BUILD_CACHE[flags]

    import concourse.bass as bass
    import concourse.bacc as bacc
    import concourse.mybir as mybir
    import concourse.tile as tile

    use_bq, use_bk, use_bv, use_qnw, use_qnb, use_knw, use_knb = flags
    f32 = mybir.dt.float32
    bf16 = mybir.dt.bfloat16
    AF = mybir.ActivationFunctionType
    MUL = mybir.AluOpType.mult
    SUB = mybir.AluOpType.subtract

    nc = bacc.Bacc("TRN2", target_bir_lowering=False, debug=False,
                   num_devices=N_CORES)

    x_d = nc.dram_tensor("x", [N, D], f32, kind="ExternalInput")
    # weights pre-rearranged on host to [128, ...] so device DMA loads are
    # fully contiguous (128 descriptors instead of 1024)
    wq_d = nc.dram_tensor("wq", [128, 8 * IL], bf16, kind="ExternalInput")
    wk_d = nc.dram_tensor("wk", [128, 8 * IL], bf16, kind="ExternalInput")
    wv_d = nc.dram_tensor("wv", [128, 8 * IL], bf16, kind="ExternalInput")
    wo_d = nc.dram_tensor("wo", [128, 4 * D], bf16, kind="ExternalInput")
    out_d = nc.dram_tensor("out", [N, D], bf16, kind="ExternalOutput")
    extra_d = {}
    for name, used in (("bq", use_bq), ("bk", use_bk), ("bv", use_bv),
                       ("qnw", use_qnw), ("qnb", use_qnb),
                       ("knw", use_knw), ("knb", use_knb)):
        if used:
            extra_d[name] = nc.dram_tensor(name, [1, IL], f32,
                                           kind="ExternalInput")

    with tile.TileContext(nc) as tc:
        with tc.tile_pool(name="const", bufs=1) as constp, \
             tc.tile_pool(name="w", bufs=1) as wpool, \
             tc.tile_pool(name="x", bufs=8) as xpool, \
             tc.tile_pool(name="xr", bufs=3) as xrpool, \
             tc.tile_pool(name="xt", bufs=2) as xtpool, \
             tc.tile_pool(name="qkln", bufs=3) as qklnp, \
             tc.tile_pool(name="qkt", bufs=1) as qktp, \
             tc.tile_pool(name="v", bufs=1) as vpool, \
             tc.tile_pool(name="ot", bufs=1) as otpool, \
             tc.tile_pool(name="pt", bufs=6) as ptpool, \
             tc.tile_pool(name="osb", bufs=2) as osbp, \
             tc.tile_pool(name="small", bufs=4) as small, \
             tc.tile_pool(name="ps", bufs=1, space="PSUM") as psp:

            # Preload the one ACT table that covers every function we use
            # (exp, ln, identity, copy) so the compiler's per-activation
            # greedy table choice never reloads mid-kernel.
            from concourse.hw_specs import get_activation_tables
            _tabs = list(get_activation_tables(nc.m.arch))
            _tab_id = _tabs.index("natural_log_exp_and_others")
            nc.scalar.add_instruction(mybir.InstLoadActFuncSet(
                name=nc.get_next_instruction_name(),
                act_func_set_id=_tab_id, ins=[], outs=[]))

            # ---- constants ----
            eps_t = constp.tile([128, 1], f32, tag="eps")
            nc.vector.memset(eps_t, EPS)
            # causal triangle: keep where q-col >= key-row
            tri = constp.tile([128, 128], bf16, tag="tri")
            nc.gpsimd.memset(tri, 1.0)
            nc.gpsimd.affine_select(
                out=tri, in_=tri, compare_op=mybir.AluOpType.is_ge,
                fill=0.0, base=0, channel_multiplier=-1, pattern=[[1, 128]])
            onesc = None
            if use_bq or use_bk or use_bv:
                onesc_f = small.tile([1, 128], f32, tag="onescf", bufs=1)
                nc.vector.memset(onesc_f, 1.0)
                onesc = constp.tile([1, 128], bf16, tag="onesc")
                nc.vector.tensor_copy(onesc, onesc_f)

            extra_sb = {}
            for name in ("bq", "bk", "bv"):
                if name in extra_d:
                    t = constp.tile([1, IL], bf16, tag=name)
                    tf = constp.tile([1, IL], f32, tag=name + "f")
                    nc.sync.dma_start(out=tf, in_=extra_d[name].ap())
                    nc.vector.tensor_copy(t, tf)
                    extra_sb[name] = t
            for name in ("qnw", "qnb", "knw", "knb"):
                if name in extra_d:
                    row = constp.tile([1, IL], f32, tag=name + "r")
                    nc.sync.dma_start(out=row, in_=extra_d[name].ap())
                    t = constp.tile([128, IL], f32, tag=name)
                    nc.gpsimd.partition_broadcast(t, row)
                    extra_sb[name] = t

            # ---- persistent tiles ----
            # qkt_g: chunks 0-3 = Q^T (dim1=hp, partition=sub*64+dh),
            #        chunks 4-7 = K^T
            qkt_g = [qktp.tile([128, 8, 512], bf16, tag=f"qkt{g}",
                               name=f"qkt_g{g}") for g in range(TG)]
            v_g = [vpool.tile([128, 4, HL, 65], bf16, tag=f"v{g}",
                              name=f"v_g{g}") for g in range(TG)]
            for g in range(TG):
                nc.gpsimd.memset(v_g[g][:, :, :, 64:65], 1.0)
            ot_g = [otpool.tile([128, 4, 512], bf16, tag=f"ot{g}",
                                name=f"ot_g{g}") for g in range(QC)]

            wq_sb = wpool.tile([128, 8, IL], bf16, tag="wq")
            wk_sb = wpool.tile([128, 8, IL], bf16, tag="wk")
            wv_sb = wpool.tile([128, 8, IL], bf16, tag="wv")
            wo_sb = wpool.tile([128, 4, D], bf16, tag="wo")
            w_of = {"q": wq_sb, "k": wk_sb, "v": wv_sb}

            state = {}

            # =================== A units ===================
            def x_load(g, t):
                x_t = xpool.tile([128, D], f32, tag="x", bufs=8,
                                 name=f"x_{g}_{t}")
                tci = g * 4 + t
                nc.sync.dma_start(
                    out=x_t, in_=x_d.ap()[tci * 128:(tci + 1) * 128, :])
                state[("x", g, t)] = x_t

            def a_ln_stats(g, t):
                if g + 2 < TG:
                    x_load(g + 2, t)
                x_t = state[("x", g, t)]
                bn = small.tile([128, 2, 6], f32, tag="bn", bufs=2,
                                name=f"bn_{g}_{t}")
                nc.vector.bn_stats(bn[:, 0, :], x_t[:, 0:512])
                nc.vector.bn_stats(bn[:, 1, :], x_t[:, 512:1024])
                mv = small.tile([128, 2], f32, tag="mv", bufs=3,
                                name=f"mv_{g}_{t}")
                nc.vector.bn_aggr(mv, bn)
                state[("mv", g, t)] = mv

            def a_ln_apply(g, t):
                x_t = state.pop(("x", g, t))
                mv = state.pop(("mv", g, t))
                # rstd = exp(-0.5 * ln(var + eps))
                nc.scalar.activation(out=mv[:, 1:2], in_=mv[:, 1:2],
                                     func=AF.Ln, bias=eps_t, scale=1.0)
                nc.scalar.activation(out=mv[:, 1:2], in_=mv[:, 1:2],
                                     func=AF.Exp, scale=-0.5)
                x_r = xrpool.tile([128, D], bf16, tag="xr", bufs=3,
                                  name=f"xr_{g}_{t}")
                # x_r = (x - mean) * rstd in one DVE op
                nc.vector.scalar_tensor_tensor(
                    out=x_r, in0=x_t, scalar=mv[:, 0:1], op0=SUB,
                    in1=mv[:, 1:2].broadcast_to([128, D]), op1=MUL)
                if t == 0:
                    state[("xt", g)] = xtpool.tile(
                        [128, 8, 512], bf16, tag="xt", bufs=2,
                        name=f"xt_{g}")
                nc.sync.dma_start(
                    out=state[("xt", g)][:, :, t * 128:(t + 1) * 128],
                    in_=x_r, transpose=True)

            def a_mm(g, t, proj):
                xt = state[("xt", g)]
                ps = psp.tile([128, 512], f32, tag="acc", bufs=2,
                              name=f"ps_{proj}_{g}_{t}")
                bias_sb = extra_sb.get("b" + proj)
                for dc in range(8):
                    nc.tensor.matmul(
                        ps, lhsT=xt[:, dc, t * 128:(t + 1) * 128],
                        rhs=w_of[proj][:, dc, :],
                        start=(dc == 0),
                        stop=(dc == 7 and bias_sb is None))
                if bias_sb is not None:
                    nc.tensor.matmul(ps, lhsT=onesc, rhs=bias_sb,
                                     start=False, stop=True)
                if proj == "v":
                    nc.vector.tensor_copy(
                        v_g[g][:, t, :, 0:64],
                        ps.rearrange("p (h d) -> p h d", h=HL))
                    return
                # free the PSUM slot in the same unit
                qraw = small.tile([128, 512], bf16, tag="qraw", bufs=4,
                                  name=f"qraw_{proj}_{g}_{t}")
                nc.vector.tensor_copy(qraw, ps)
                state[("qraw", proj, g, t)] = qraw

            def a_chain1(g, t):
                ss = small.tile([128, 2, HL], f32, tag="ss", bufs=2,
                                name=f"ss_{g}_{t}")
                for half, proj in enumerate(("q", "k")):
                    qraw = state[("qraw", proj, g, t)]
                    sq = small.tile([128, 512], bf16, tag="sq", bufs=2,
                                    name=f"sq_{proj}_{g}_{t}")
                    nc.gpsimd.tensor_mul(sq, qraw, qraw)
                    nc.vector.reduce_sum(
                        ss[:, half, :],
                        sq.rearrange("p (h d) -> p h d", h=HL),
                        axis=mybir.AxisListType.X)
                state[("ss", g, t)] = ss

            def a_chain2(g, t):
                ss = state.pop(("ss", g, t))
                # joint rstd for q and k: exp(-0.5*ln(ss/Dh + eps))
                nc.scalar.activation(out=ss, in_=ss, func=AF.Ln,
                                     bias=eps_t, scale=1.0 / Dh)
                nc.scalar.activation(out=ss, in_=ss, func=AF.Exp,
                                     scale=-0.5)
                qkln = qklnp.tile([128, 2, 512], bf16, tag="qkln", bufs=3,
                                  name=f"qkln_{g}_{t}")
                for half, proj in enumerate(("q", "k")):
                    qraw = state.pop(("qraw", proj, g, t))
                    nc.vector.scalar_tensor_tensor(
                        out=qkln[:, half, :].rearrange(
                            "p (h d) -> p h d", h=HL),
                        in0=qraw.rearrange("p (h d) -> p h d", h=HL),
                        scalar=1.0,
                        in1=ss[:, half, :].broadcast_to([128, HL, Dh]),
                        op0=MUL, op1=MUL)
                    if extra_sb.get(proj + "nw") is not None:
                        nc.vector.tensor_mul(
                            qkln[:, half, :], qkln[:, half, :],
                            extra_sb[proj + "nw"])
                    if extra_sb.get(proj + "nb") is not None:
                        nc.vector.tensor_add(
                            qkln[:, half, :], qkln[:, half, :],
                            extra_sb[proj + "nb"])
                nc.sync.dma_start(
                    out=qkt_g[g][:, :, t * 128:(t + 1) * 128],
                    in_=qkln.rearrange("p a b -> p (a b)"),
                    transpose=True)

            def units_a_all():
                # one globally software-pipelined A stream over all 16
                # tiles: stats/apply run ~1-2 tiles ahead of the matmuls,
                # chain2 one tile behind, so every instruction is ready
                # when its in-order engine queue reaches it and there are
                # no group-boundary stalls.
                seq = [(g, t) for g in range(TG) for t in range(4)]
                n = len(seq)
                S = lambda i: (lambda: a_ln_stats(*seq[i]))
                A_ = lambda i: (lambda: a_ln_apply(*seq[i]))
                Q = lambda i: (lambda: a_mm(*seq[i], "q"))
                K = lambda i: (lambda: a_mm(*seq[i], "k"))
                V = lambda i: (lambda: a_mm(*seq[i], "v"))
                C1 = lambda i: (lambda: a_chain1(*seq[i]))
                C2 = lambda i: (lambda: a_chain2(*seq[i]))
                us = [S(0), S(1), A_(0)]
                for i in range(n):
                    if i + 2 < n:
                        us.append(S(i + 2))
                    if i + 1 < n:
                        us.append(A_(i + 1))
                    us.extend([Q(i), K(i), V(i), C1(i)])
                    if i >= 1:
                        us.append(C2(i - 1))
                us.append(C2(n - 1))
                return us

            # =================== B units ===================
            def emit_pv(qc, hp, hold):
                pts, kc0 = hold["prev"]
                nkc = 4 * (qc + 1)
                for sub in range(2):
                    h = 2 * hp + sub
                    for i in range(2):
                        kc = kc0 + i
                        d = kc * 128 - qc * 512
                        q0c = d if d > 0 else 0
                        nc.tensor.matmul(
                            hold["ps_o"][sub][:, q0c:512],
                            lhsT=v_g[kc // 4][:, kc % 4, h, :],
                            rhs=pts[sub][:, i, q0c:512],
                            start=(kc == 0), stop=(kc == nkc - 1))

            def b_unit(qc, hp, kcg, hold):
                if kcg == 0:
                    hold["ps_o"] = [
                        psp.tile([65, 512], f32, tag="o", bufs=2,
                                 name=f"o_{qc}_{hp}_{s}") for s in range(2)]
                    hold["prev"] = None
                kc0 = 2 * kcg
                d0 = kc0 * 128 - qc * 512     # >= 0 iff diagonal pair
                q0p = 256 if d0 >= 256 else 0
                pts = []
                for sub in range(2):
                    r0 = 64 * sub
                    ps_s = psp.tile([128, 2, 512], f32, tag="s", bufs=2,
                                    name=f"s_{qc}_{hp}_{kcg}_{sub}")
                    for i in range(2):
                        kc = kc0 + i
                        d = kc * 128 - qc * 512
                        q0c = d if d > 0 else 0
                        nc.tensor.matmul(
                            ps_s[:, i, q0c:512],
                            lhsT=qkt_g[kc // 4][
                                r0:r0 + 64, 4 + hp,
                                (kc % 4) * 128:(kc % 4 + 1) * 128],
                            rhs=qkt_g[qc][r0:r0 + 64, hp, q0c:512],
                            start=True, stop=True)
                    pt = ptpool.tile([128, 2, 512], bf16, tag="pt", bufs=6,
                                     name=f"pt_{qc}_{hp}_{kcg}_{sub}")
                    nc.scalar.activation(
                        out=pt[:, :, q0p:512], in_=ps_s[:, :, q0p:512],
                        func=AF.Exp, scale=SCALE)
                    if d0 >= 0:
                        # batched tri-mask over both diag chunks of the pair
                        pt_ap = bass.AP(
                            tensor=pt.tensor, offset=pt.offset + d0,
                            ap=[pt.ap[0], [512 + 128, 2], [1, 128]])
                        tri_b = bass.AP(
                            tensor=tri.tensor, offset=tri.offset,
                            ap=[tri.ap[0], [0, 2], [1, 128]])
                        nc.vector.tensor_mul(pt_ap, pt_ap, tri_b)
                    pts.append(pt)
                if hold["prev"] is not None:
                    emit_pv(qc, hp, hold)
                hold["prev"] = (pts, kc0)

            def b_tail_a(qc, hp, hold):
                # last PV + copy out of PSUM (releases the ps_o ring fast)
                emit_pv(qc, hp, hold)
                for sub in range(2):
                    ps_o = hold["ps_o"][sub]
                    rf = small.tile([1, 512], f32, tag="rf", bufs=4,
                                    name=f"rf_{qc}_{hp}_{sub}")
                    nc.vector.tensor_copy(rf, ps_o[64:65, :])
                    ob = small.tile([64, 512], bf16, tag="ob", bufs=4,
                                    name=f"ob_{qc}_{hp}_{sub}")
                    nc.vector.tensor_copy(ob, ps_o[0:64, :])
                    state[("tail", qc, hp, sub)] = (rf, ob)

            def b_tail_b(qc, hp):
                # lagged normalize: everything reads SBUF, nothing blocks
                for sub in range(2):
                    r0 = 64 * sub
                    rf, ob = state.pop(("tail", qc, hp, sub))
                    nc.vector.reciprocal_approx_fast(rf, rf)
                    rb = small.tile([128, 512], f32, tag="rb", bufs=2,
                                    name=f"rb_{qc}_{hp}_{sub}")
                    nc.gpsimd.partition_broadcast(rb, rf)
                    nc.vector.scalar_tensor_tensor(
                        out=ot_g[qc][r0:r0 + 64, hp, :],
                        in0=ob, scalar=1.0, in1=rb[0:64, :],
                        op0=MUL, op1=MUL)

            def units_b(qc):
                us = []
                pend = []
                for hp in range(4):
                    hold = {}
                    for kcg in range(2 * (qc + 1)):
                        us.append(
                            lambda qc=qc, hp=hp, kcg=kcg, hold=hold:
                            b_unit(qc, hp, kcg, hold))
                        if pend and kcg == 1:
                            us.append(pend.pop(0))
                    us.append(lambda qc=qc, hp=hp, hold=hold:
                              b_tail_a(qc, hp, hold))
                    pend.append(lambda qc=qc, hp=hp: b_tail_b(qc, hp))
                us.extend(pend)
                return us

            # =================== C units ===================
            def c_mm(qc, t):
                tci = qc * 4 + t
                osb = osbp.tile([128, 2, 512], bf16, tag="osb", bufs=2,
                                name=f"osb_{tci}")
                pss = []
                for dch in range(2):
                    ps = psp.tile([128, 512], f32, tag="acc", bufs=2,
                                  name=f"ps_c_{tci}_{dch}")
                    for m in range(4):
                        nc.tensor.matmul(
                            ps, lhsT=ot_g[qc][:, m, t * 128:(t + 1) * 128],
                            rhs=wo_sb[:, m, dch * 512:(dch + 1) * 512],
                            start=(m == 0), stop=(m == 3))
                    pss.append(ps)
                state[("c", tci)] = (osb, pss)

            def c_fin(qc, t):
                tci = qc * 4 + t
                osb, pss = state.pop(("c", tci))
                nc.vector.tensor_copy(osb[:, 0, :], pss[0])
                nc.vector.tensor_copy(osb[:, 1, :], pss[1])
                nc.sync.dma_start(
                    out=out_d.ap()[tci * 128:(tci + 1) * 128, :],
                    in_=osb.rearrange("p a b -> p (a b)"))

            def units_c(qc):
                us = []
                for t in range(4):
                    us.append(lambda qc=qc, t=t: c_mm(qc, t))
                    us.append(lambda qc=qc, t=t: c_fin(qc, t))
                return us

            # =================== schedule ===================
            x_t00 = xpool.tile([128, D], f32, tag="x", bufs=8,
                               name="x_0_0")
            nc.sync.dma_start(out=x_t00[:, 0:512],
                              in_=x_d.ap()[0:128, 0:512])
            nc.sync.dma_start(out=x_t00[:, 512:1024],
                              in_=x_d.ap()[0:128, 512:1024])
            state[("x", 0, 0)] = x_t00
            x_load(0, 1)
            nc.sync.dma_start(out=wq_sb, in_=wq_d.ap())
            x_load(0, 2)
            x_load(0, 3)
            nc.sync.dma_start(out=wk_sb, in_=wk_d.ap())
            x_load(1, 0)
            x_load(1, 1)
            nc.sync.dma_start(out=wv_sb, in_=wv_d.ap())
            x_load(1, 2)
            x_load(1, 3)
            nc.sync.dma_start(out=wo_sb, in_=wo_d.ap())

            for u in units_a_all():
                u()

            def interleave(primary, fillers, front=0.85):
                """Emit primary units with fillers spread between them,
                front-biased so fillers finish by `front` fraction of the
                primary stream."""
                np_, nf = len(primary), len(fillers)
                fi = 0
                for j, u in enumerate(primary):
                    u()
                    want = min(nf, int(nf * (j + 1) / max(1, front * np_)))
                    while fi < want:
                        fillers[fi]()
                        fi += 1
                while fi < nf:
                    fillers[fi]()
                    fi += 1

            interleave(units_b(0), [])
            interleave(units_b(1), units_c(0))
            interleave(units_b(2), units_c(1))
            interleave(units_b(3), units_c(2), front=1.0)
            for u in units_c(3):
                u()

    nc.compile()
    _BUILD_CACHE[flags] = nc
    return nc


def kernel(**inputs):
    x = np.ascontiguousarray(np.asarray(inputs["x"], np.float32))
    ln_w = np.asarray(inputs["ln_w"], np.float32)
    ln_b = np.asarray(inputs["ln_b"], np.float32)
    Wq = np.asarray(inputs["Wq"], np.float32)
    Wk = np.asarray(inputs["Wk"], np.float32)
    Wv = np.asarray(inputs["Wv"], np.float32)
    qn_w = np.asarray(inputs["qn_w"], np.float32)
    qn_b = np.asarray(inputs["qn_b"], np.float32)
    kn_w = np.asarray(inputs["kn_w"], np.float32)
    kn_b = np.asarray(inputs["kn_b"], np.float32)
    Wo = np.asarray(inputs["Wo"], np.float32)
    bo = np.asarray(inputs["bo"], np.float32)

    # ---- host-side weight folding ----
    def fold(W):
        return ln_w[:, None] * W, ln_b @ W

    W1q, bq = fold(Wq)
    W1k, bk = fold(Wk)
    W1v, bv = fold(Wv)

    def center(W, b):
        W3 = W.reshape(D, H, Dh)
        W3 = W3 - W3.mean(-1, keepdims=True)
        b3 = b.reshape(H, Dh)
        b3 = b3 - b3.mean(-1, keepdims=True)
        return np.ascontiguousarray(W3.reshape(D, H * Dh)), b3.reshape(H * Dh)

    W1q, bq = center(W1q, bq)
    W1k, bk = center(W1k, bk)

    flags = (
        bool(np.any(bq)), bool(np.any(bk)), bool(np.any(bv)),
        not np.all(qn_w == 1.0), bool(np.any(qn_b)),
        not np.all(kn_w == 1.0), bool(np.any(kn_b)),
    )
    nc = _build_program(flags)

    def prearrange_w(W, nchunk):
        # [nchunk*128, F] -> [128, nchunk*F] with w_sb[p, c, f] = W[c*128+p, f]
        Wb = W.astype(ml_dtypes.bfloat16)
        F = Wb.shape[1]
        return np.ascontiguousarray(
            Wb.reshape(nchunk, 128, F).transpose(1, 0, 2).reshape(
                128, nchunk * F))

    wo_bf = Wo.astype(ml_dtypes.bfloat16)
    in_maps = []
    for c in range(N_CORES):
        b, g = c // 2, c % 2
        sl = slice(IL * g, IL * (g + 1))
        m = {
            "x": x[b],
            "wq": prearrange_w(W1q[:, sl], 8),
            "wk": prearrange_w(W1k[:, sl], 8),
            "wv": prearrange_w(W1v[:, sl], 8),
            "wo": prearrange_w(Wo[sl, :], 4),
        }
        if flags[0]:
            m["bq"] = np.ascontiguousarray(bq[None, sl])
        if flags[1]:
            m["bk"] = np.ascontiguousarray(bk[None, sl])
        if flags[2]:
            m["bv"] = np.ascontiguousarray(bv[None, sl])
        if flags[3]:
            m["qnw"] = np.ascontiguousarray(np.tile(qn_w, HL)[None, :])
        if flags[4]:
            m["qnb"] = np.ascontiguousarray(np.tile(qn_b, HL)[None, :])
        if flags[5]:
            m["knw"] = np.ascontiguousarray(np.tile(kn_w, HL)[None, :])
        if flags[6]:
            m["knb"] = np.ascontiguousarray(np.tile(kn_b, HL)[None, :])
        in_maps.append(m)

    from concourse.bass_utils import run_bass_kernel_spmd

    trace = _install_ntff_hook_shim() and \
        os.environ.get("KERNEL_NO_TRACE", "0") != "1"
    try:
        res = run_bass_kernel_spmd(
            nc, in_maps, core_ids=list(range(N_CORES)), trace=trace)
    except Exception:
        if not trace:
            raise
        res = run_bass_kernel_spmd(
            nc, in_maps, core_ids=list(range(N_CORES)), trace=False)
    globals()["LAST_RESULT"] = res
    if res.exec_time_ns is not None:
        print(f"HW exec time: {res.exec_time_ns} ns")

    out = np.zeros((B, N, D), np.float32)
    for b in range(B):
        out[b] = (np.asarray(res.results[2 * b]["out"], np.float32) +
                  np.asarray(res.results[2 * b + 1]["out"], np.float32))
    out += bo
    return out
